# revision 9
# baseline (speedup 1.0000x reference)
"""TRN2 Bass/Tile kernel: Llama attention block (B=1, S=2048, D=2048, H=16, causal).

Sharding: tensor-parallel over heads. 16 heads / 8 cores = 2 heads per core.
Wq/Wk/Wv column-sharded (256 dims per core), Wo column-sharded on the output
side after an AllGather of the per-core attention outputs.

Per-core dataflow (all matmuls bf16 operands, fp32 PSUM accumulate):
  - host passes X.T so the contraction dim is on partitions everywhere
  - qT/kT computed in [hd, S] layout, v in natural [S, hd] layout
  - RoPE: rotate-half via a +-1 permutation matmul, then q' = q*cos + rot*sin
  - attention computes scoresT[t, sq] = kT_tile.T @ qT, exp on ScalarE (no
    max-subtraction: |scaled scores| < 5 for this data), causal mask by
    multiplying a 0/1 staircase, probs consumed directly as the moving
    operand of the v-matmul -> attn.T[hd, sq] with zero transposes
  - softmax denominators: running DVE sum over t-tiles, then a ones-matmul
    broadcasts the partition-sum to all partitions; DVE reciprocal+mul
  - per-head AllGather of attn.T into all cores, column-sharded Wo matmul
"""

import os
import sys

import numpy as np

for _p in ("/opt/trn_rl_repo",):
    if _p not in sys.path and os.path.isdir(_p):
        sys.path.insert(0, _p)

P = 128            # SBUF partitions
S = 2048           # sequence length
D = 2048           # hidden dim
NCORES = 8
DC = D // NCORES   # 256 = head-dims per core
HPC = 2            # heads per core
HD = 128           # head dim
KT = D // P        # 16 contraction tiles
SQW = 512          # sq tile width (moving free dim)
NSQ = S // SQW     # 4
NT = S // P        # 16 t tiles
SM = float(1.0 / np.sqrt(HD))

_NC_CACHE = {}
LAST_RESULTS = None


def _build_nc(reps=1):
    import concourse.bacc as bacc
    import concourse.mybir as mybir
    from concourse import tile

    fp32 = mybir.dt.float32
    bf16 = mybir.dt.bfloat16
    Exp = mybir.ActivationFunctionType.Exp

    nc = bacc.Bacc("TRN2", num_devices=NCORES, debug=False)

    xt = nc.dram_tensor("xt", [D, S], bf16, kind="ExternalInput")
    wq = nc.dram_tensor("wq", [D, DC], bf16, kind="ExternalInput")
    wk = nc.dram_tensor("wk", [D, DC], bf16, kind="ExternalInput")
    wv = nc.dram_tensor("wv", [D, DC], bf16, kind="ExternalInput")
    wo = nc.dram_tensor("wo", [D, DC], bf16, kind="ExternalInput")
    cost = nc.dram_tensor("cost", [HD, S], fp32, kind="ExternalInput")
    sint = nc.dram_tensor("sint", [HD, S], fp32, kind="ExternalInput")
    rt = nc.dram_tensor("rt", [HD, HD], bf16, kind="ExternalInput")
    msk = nc.dram_tensor("msk", [P, 896], bf16, kind="ExternalInput")
    ones = nc.dram_tensor("ones", [P, P], bf16, kind="ExternalInput")
    out = nc.dram_tensor("out", [S, DC], fp32, kind="ExternalOutput")

    xt_r = xt.rearrange("(k p) s -> p k s", p=P)
    wq_r = wq.rearrange("(k p) d -> p k d", p=P)
    wk_r = wk.rearrange("(k p) d -> p k d", p=P)
    wv_r = wv.rearrange("(k p) d -> p k d", p=P)
    wo_r = wo.rearrange("(k p) d -> p k d", p=P)
    out_r = out.rearrange("(m p) d -> m p d", p=P)

    def emit_body(tc, rep):
        r = f"r{rep}"
        with (
            tc.tile_pool(name=f"const{r}", bufs=1) as const,
            tc.tile_pool(name=f"acts{r}", bufs=1) as acts,
            tc.tile_pool(name=f"work{r}", bufs=2) as work,
            tc.tile_pool(name=f"ps{r}", bufs=2, space="PSUM") as ps,
            tc.tile_pool(name=f"dram{r}", bufs=1, space="DRAM") as dram,
        ):
            # ---- constants / weights ----
            wq_sb = const.tile([P, KT, DC], bf16, name=f"wq_sb{r}")
            wk_sb = const.tile([P, KT, DC], bf16, name=f"wk_sb{r}")
            wv_sb = const.tile([P, KT, DC], bf16, name=f"wv_sb{r}")
            wo_sb = const.tile([P, KT, DC], bf16, name=f"wo_sb{r}")
            cos_sb = const.tile([HD, S], fp32, name=f"cos_sb{r}")
            sin_sb = const.tile([HD, S], fp32, name=f"sin_sb{r}")
            rt_sb = const.tile([HD, HD], bf16, name=f"rt_sb{r}")
            msk_sb = const.tile([P, 896], bf16, name=f"msk_sb{r}")
            ones_sb = const.tile([P, P], bf16, name=f"ones_sb{r}")
            nc.sync.dma_start(wq_sb[:], wq_r)
            nc.sync.dma_start(wk_sb[:], wk_r)
            nc.sync.dma_start(wv_sb[:], wv_r)
            nc.sync.dma_start(wo_sb[:], wo_r)
            nc.sync.dma_start(cos_sb[:], cost[:])
            nc.sync.dma_start(sin_sb[:], sint[:])
            nc.sync.dma_start(rt_sb[:], rt[:])
            nc.sync.dma_start(msk_sb[:], msk[:])
            nc.sync.dma_start(ones_sb[:], ones[:])

            # ---- persistent activations ----
            qfin = acts.tile([HD, HPC, S], bf16, name=f"qfin{r}")
            kfin = acts.tile([HD, HPC, S], bf16, name=f"kfin{r}")
            v_sb = acts.tile([P, NT, DC], bf16, name=f"v_sb{r}")
            attnT = acts.tile([HD, HPC, S], bf16, name=f"attnT{r}")

            # collective buffers (one AllGather per head so head-0's gather
            # overlaps head-1's attention)
            cc_in = [dram.tile([HD, S], bf16, name=f"ccin{h}{r}")
                     for h in range(HPC)]
            cc_out = [
                dram.tile([NCORES, HD, S], bf16, addr_space="Shared",
                          name=f"ccout{h}{r}")
                for h in range(HPC)
            ]

            with tc.tile_pool(name=f"xtp{r}", bufs=1) as xtp:
                xt_sb = xtp.tile([P, KT, S], bf16, name=f"xt_sb{r}")
                qraw = xtp.tile([HD, HPC, S], bf16, name=f"qraw{r}")
                kraw = xtp.tile([HD, HPC, S], bf16, name=f"kraw{r}")
                for kt in range(KT):
                    nc.sync.dma_start(xt_sb[:, kt, :], xt_r[:, kt, :])

                # ---- q/k projections (transposed layout) ----
                for w_sb, raw in ((wq_sb, qraw), (wk_sb, kraw)):
                    for m in range(HPC):
                        for n in range(NSQ):
                            pp = ps.tile([P, SQW], fp32, tag="proj", bufs=2,
                                         name="pp")
                            for kt in range(KT):
                                nc.tensor.matmul(
                                    pp[:],
                                    w_sb[:, kt, m * HD:(m + 1) * HD],
                                    xt_sb[:, kt, n * SQW:(n + 1) * SQW],
                                    start=(kt == 0),
                                    stop=(kt == KT - 1),
                                )
                            nc.vector.tensor_copy(
                                raw[:, m, n * SQW:(n + 1) * SQW], pp[:]
                            )

                # ---- v projection (natural layout) ----
                for m in range(NT):
                    pv = ps.tile([P, DC], fp32, tag="proj", bufs=2, name="pv")
                    for kt in range(KT):
                        nc.tensor.matmul(
                            pv[:],
                            xt_sb[:, kt, m * P:(m + 1) * P],
                            wv_sb[:, kt, :],
                            start=(kt == 0),
                            stop=(kt == KT - 1),
                        )
                    nc.vector.tensor_copy(v_sb[:, m, :], pv[:])

                # ---- RoPE ----
                for raw, fin in ((qraw, qfin), (kraw, kfin)):
                    for m in range(HPC):
                        for n in range(NSQ):
                            nsl = slice(n * SQW, (n + 1) * SQW)
                            pr = ps.tile([P, SQW], fp32, tag="proj", bufs=2,
                                         name="pr")
                            nc.tensor.matmul(
                                pr[:], rt_sb[:], raw[:, m, nsl],
                                start=True, stop=True,
                            )
                            t1 = work.tile([P, SQW], fp32, tag="t1", bufs=2,
                                           name="t1")
                            t2 = work.tile([P, SQW], fp32, tag="t2", bufs=2,
                                           name="t2")
                            nc.vector.tensor_mul(t1[:], raw[:, m, nsl],
                                                 cos_sb[:, nsl])
                            nc.vector.tensor_mul(t2[:], pr[:], sin_sb[:, nsl])
                            nc.vector.tensor_add(fin[:, m, nsl], t1[:], t2[:])

            # ---- attention ----
            for h in range(HPC):
                hsl = slice(h * HD, (h + 1) * HD)
                for i in range(NSQ):
                    sq = slice(i * SQW, (i + 1) * SQW)
                    njt = 4 * i + 4
                    pa = ps.tile([HD, SQW], fp32, tag="attn", bufs=2, name="pa")
                    ssum = work.tile([P, SQW], fp32, tag="ssum", bufs=2,
                                     name="ssum")
                    for j in range(njt):
                        psc = ps.tile([P, SQW], fp32, tag="sc", bufs=2,
                                      name="psc")
                        nc.tensor.matmul(
                            psc[:],
                            kfin[:, h, j * P:(j + 1) * P],
                            qfin[:, h, sq],
                            start=True, stop=True,
                        )
                        e = work.tile([P, SQW], bf16, tag="e", bufs=3, name="e")
                        nc.scalar.activation(e[:], psc[:], Exp, scale=SM)
                        m = j - 4 * i
                        if m >= 0:
                            em = work.tile([P, SQW], bf16, tag="em", bufs=3,
                                           name="em")
                            nc.vector.tensor_mul(
                                em[:], e[:],
                                msk_sb[:, 384 - 128 * m: 896 - 128 * m]
                            )
                            e = em
                        if j == 0:
                            nc.vector.tensor_copy(ssum[:], e[:])
                        else:
                            nc.vector.tensor_add(ssum[:], ssum[:], e[:])
                        nc.tensor.matmul(
                            pa[:],
                            v_sb[:, j, hsl],
                            e[:],
                            start=(j == 0),
                            stop=(j == njt - 1),
                        )
                    sb = work.tile([P, SQW], bf16, tag="ssumb", bufs=2,
                                   name="sb")
                    nc.vector.tensor_copy(sb[:], ssum[:])
                    pl = ps.tile([P, SQW], fp32, tag="l", bufs=2, name="pl")
                    nc.tensor.matmul(pl[:], ones_sb[:], sb[:],
                                     start=True, stop=True)
                    rec = work.tile([P, SQW], fp32, tag="rec", bufs=2,
                                    name="rec")
                    nc.vector.reciprocal(rec[:], pl[:])
                    nc.vector.tensor_mul(attnT[:, h, sq], pa[:], rec[:])

                # ship this head's attn.T to every core
                nc.sync.dma_start(cc_in[h][:], attnT[:, h, :])
                nc.gpsimd.collective_compute(
                    "AllGather",
                    mybir.AluOpType.bypass,
                    replica_groups=[list(range(NCORES))],
                    ins=[cc_in[h][:].opt()],
                    outs=[cc_out[h][:].opt()],
                )

            # ---- output projection (columns DC*c .. DC*c+DC) ----
            with tc.tile_pool(name=f"gath{r}", bufs=1) as gath:
                ag_sb = gath.tile([P, KT, S], bf16, name=f"ag_sb{r}")
                out_sb = gath.tile([P, NT, DC], fp32, name=f"out_sb{r}")
                # cc_out[h][c] holds dattn rows 256c+128h..+128 -> kt = 2c+h
                for c in range(NCORES):
                    for h in range(HPC):
                        nc.sync.dma_start(ag_sb[:, 2 * c + h, :], cc_out[h][c])
                # contract head-0 k-tiles first: they are gathered earlier
                kt_order = [2 * c for c in range(NCORES)] + \
                           [2 * c + 1 for c in range(NCORES)]
                for m in range(NT):
                    po = ps.tile([P, DC], fp32, tag="proj", bufs=2, name="po")
                    for ki, kt in enumerate(kt_order):
                        nc.tensor.matmul(
                            po[:],
                            ag_sb[:, kt, m * P:(m + 1) * P],
                            wo_sb[:, kt, :],
                            start=(ki == 0),
                            stop=(ki == KT - 1),
                        )
                    nc.vector.tensor_copy(out_sb[:, m, :], po[:])
                    nc.sync.dma_start(out_r[m], out_sb[:, m, :])

    with tile.TileContext(nc) as tc:
        for rep in range(reps):
            emit_body(tc, rep)

    nc.compile()
    return nc


def _get_nc(reps=1):
    key = ("nc", reps)
    if key not in _NC_CACHE:
        _NC_CACHE[key] = _build_nc(reps)
    return _NC_CACHE[key]


def _host_tables():
    import ml_dtypes

    bf = ml_dtypes.bfloat16
    inv_freq = 1.0 / (10000.0 ** (np.arange(0, HD, 2, dtype=np.float32) / HD))
    t = np.arange(S, dtype=np.float32)
    freqs = np.outer(t, inv_freq)
    emb = np.concatenate([freqs, freqs], axis=-1)        # [S, HD]
    cosT = np.ascontiguousarray(np.cos(emb).T).astype(np.float32)
    sinT = np.ascontiguousarray(np.sin(emb).T).astype(np.float32)

    rt = np.zeros((HD, HD), dtype=np.float32)
    for e in range(64):
        rt[e, e + 64] = 1.0
    for e in range(64, HD):
        rt[e, e - 64] = -1.0

    y = np.arange(896)[None, :]
    tl = np.arange(P)[:, None]
    mskM = (tl <= (y - 384)).astype(np.float32)

    ones = np.ones((P, P), dtype=np.float32)
    return cosT, sinT, rt.astype(bf), mskM.astype(bf), ones.astype(bf)


def _prep_in_maps(hidden_states, Wq, Wk, Wv, Wo):
    import ml_dtypes

    bf = ml_dtypes.bfloat16
    X = np.asarray(hidden_states, dtype=np.float32).reshape(S, D)
    Wq = np.asarray(Wq, dtype=np.float32)
    Wk = np.asarray(Wk, dtype=np.float32)
    Wv = np.asarray(Wv, dtype=np.float32)
    Wo = np.asarray(Wo, dtype=np.float32)

    XT = np.ascontiguousarray(X.T).astype(bf)
    cosT, sinT, rt, mskM, ones = _host_tables()

    in_maps = []
    for c in range(NCORES):
        sl = slice(DC * c, DC * (c + 1))
        in_maps.append({
            "xt": XT,
            "wq": np.ascontiguousarray(Wq[sl].T).astype(bf),
            "wk": np.ascontiguousarray(Wk[sl].T).astype(bf),
            "wv": np.ascontiguousarray(Wv[sl].T).astype(bf),
            "wo": np.ascontiguousarray(Wo[sl].T).astype(bf),
            "cost": cosT,
            "sint": sinT,
            "rt": rt,
            "msk": mskM,
            "ones": ones,
        })
    return in_maps


def kernel(hidden_states, Wq, Wk, Wv, Wo):
    global LAST_RESULTS
    from concourse.bass_utils import run_bass_kernel_spmd

    in_maps = _prep_in_maps(hidden_states, Wq, Wk, Wv, Wo)
    nc = _get_nc()
    res = run_bass_kernel_spmd(nc, in_maps, core_ids=list(range(NCORES)))
    LAST_RESULTS = res

    out = np.concatenate(
        [np.asarray(res.results[c]["out"]) for c in range(NCORES)], axis=1
    )
    return out.reshape(1, S, D).astype(np.float32)


# revision 17
# speedup vs baseline: 1.8024x; 1.8024x over previous
"""TRN2 Bass/Tile kernel: Llama attention block (B=1, S=2048, D=2048, H=16, causal).

Sharding: tensor-parallel over heads. 16 heads / 8 cores = 2 heads per core.
Wq/Wk/Wv column-sharded (256 dims per core), Wo column-sharded on the output
side after an AllGather of the per-core attention outputs.

Per-core dataflow (all matmuls bf16 operands, fp32 PSUM accumulate):
  - host passes X.T so the contraction dim is on partitions everywhere
  - qT/kT computed in [hd, S] layout, v in natural [S, hd] layout
  - RoPE: rotate-half via a +-1 permutation matmul, then q' = q*cos + rot*sin
  - attention computes scoresT[t, sq] = kT_tile.T @ qT, exp on ScalarE (no
    max-subtraction: |scaled scores| < 5 for this data), causal mask by
    multiplying a 0/1 staircase, probs consumed directly as the moving
    operand of the v-matmul -> attn.T[hd, sq] with zero transposes
  - softmax denominators: running DVE sum over t-tiles, then a ones-matmul
    broadcasts the partition-sum to all partitions; DVE reciprocal+mul
  - per-head AllGather of attn.T into all cores, column-sharded Wo matmul
"""

import os
import sys

import numpy as np

for _p in ("/opt/trn_rl_repo",):
    if _p not in sys.path and os.path.isdir(_p):
        sys.path.insert(0, _p)

P = 128            # SBUF partitions
S = 2048           # sequence length
D = 2048           # hidden dim
NCORES = 8
DC = D // NCORES   # 256 = head-dims per core
HPC = 2            # heads per core
HD = 128           # head dim
KT = D // P        # 16 contraction tiles
SQW = 512          # sq tile width (moving free dim)
NSQ = S // SQW     # 4
NT = S // P        # 16 t tiles
SCS = S // NCORES  # 256 output seq rows per core (sequence-parallel Wo)
SM = float(1.0 / np.sqrt(HD))

_NC_CACHE = {}
LAST_RESULTS = None


def _build_nc(reps=1):
    import concourse.bacc as bacc
    import concourse.mybir as mybir
    from concourse import tile

    fp32 = mybir.dt.float32
    bf16 = mybir.dt.bfloat16
    Exp = mybir.ActivationFunctionType.Exp

    nc = bacc.Bacc("TRN2", num_devices=NCORES, debug=False)

    xt = nc.dram_tensor("xt", [D, S], bf16, kind="ExternalInput")
    wq = nc.dram_tensor("wq", [D, DC], bf16, kind="ExternalInput")
    wk = nc.dram_tensor("wk", [D, DC], bf16, kind="ExternalInput")
    wv = nc.dram_tensor("wv", [D, DC], bf16, kind="ExternalInput")
    wo = nc.dram_tensor("wo", [D, D], bf16, kind="ExternalInput")  # full Wo.T
    cost = nc.dram_tensor("cost", [HD, S], fp32, kind="ExternalInput")
    sint = nc.dram_tensor("sint", [HD, S], fp32, kind="ExternalInput")
    rt = nc.dram_tensor("rt", [HD, HD], bf16, kind="ExternalInput")
    msk = nc.dram_tensor("msk", [P, 896], bf16, kind="ExternalInput")
    ones = nc.dram_tensor("ones", [P, P], bf16, kind="ExternalInput")
    out = nc.dram_tensor("out", [SCS, D], fp32, kind="ExternalOutput")

    xt_r = xt.rearrange("(k p) s -> p k s", p=P)
    wq_r = wq.rearrange("(k p) d -> p k d", p=P)
    wk_r = wk.rearrange("(k p) d -> p k d", p=P)
    wv_r = wv.rearrange("(k p) d -> p k d", p=P)
    wo_r = wo.rearrange("(k p) d -> p k d", p=P)
    out_r = out.rearrange("(m p) d -> m p d", p=P)

    def emit_body(tc, rep):
        r = f"r{rep}"
        with (
            tc.tile_pool(name=f"const{r}", bufs=1) as const,
            tc.tile_pool(name=f"acts{r}", bufs=1) as acts,
            tc.tile_pool(name=f"work{r}", bufs=2) as work,
            tc.tile_pool(name=f"ps{r}", bufs=2, space="PSUM") as ps,
            tc.tile_pool(name=f"dram{r}", bufs=1, space="DRAM") as dram,
        ):
            # ---- constants / weights ----
            wq_sb = const.tile([P, KT, DC], bf16, name=f"wq_sb{r}")
            wk_sb = const.tile([P, KT, DC], bf16, name=f"wk_sb{r}")
            wv_sb = const.tile([P, KT, DC], bf16, name=f"wv_sb{r}")
            cos_sb = const.tile([HD, S], fp32, name=f"cos_sb{r}")
            sin_sb = const.tile([HD, S], fp32, name=f"sin_sb{r}")
            rt_sb = const.tile([HD, HD], bf16, name=f"rt_sb{r}")
            msk_sb = const.tile([P, 896], bf16, name=f"msk_sb{r}")
            ones_sb = const.tile([P, P], bf16, name=f"ones_sb{r}")
            nc.sync.dma_start(wq_sb[:], wq_r)
            nc.sync.dma_start(wk_sb[:], wk_r)
            nc.sync.dma_start(wv_sb[:], wv_r)
            nc.sync.dma_start(cos_sb[:], cost[:])
            nc.sync.dma_start(sin_sb[:], sint[:])
            nc.sync.dma_start(rt_sb[:], rt[:])
            nc.sync.dma_start(msk_sb[:], msk[:])
            nc.sync.dma_start(ones_sb[:], ones[:])

            # ---- persistent activations ----
            qfin = acts.tile([HD, HPC, S], bf16, name=f"qfin{r}")
            kfin = acts.tile([HD, HPC, S], bf16, name=f"kfin{r}")
            v_sb = acts.tile([P, NT, DC], bf16, name=f"v_sb{r}")
            attnT = acts.tile([HD, HPC, S], bf16, name=f"attnT{r}")

            # AllToAll buffers: block j of a2a_in (this core's attn.T columns
            # s in [256j, 256j+256)) is sent to core j; core j then holds
            # attn.T[:, its seq slice] from every core.
            a2a_in = dram.tile([NCORES, DC, SCS], bf16, name=f"a2ain{r}")
            a2a_out = dram.tile([NCORES, DC, SCS], bf16, name=f"a2aout{r}")

            with tc.tile_pool(name=f"xtp{r}", bufs=1) as xtp:
                xt_sb = xtp.tile([P, KT, S], bf16, name=f"xt_sb{r}")
                qraw = xtp.tile([HD, HPC, S], bf16, name=f"qraw{r}")
                kraw = xtp.tile([HD, HPC, S], bf16, name=f"kraw{r}")
                for kt in range(KT):
                    nc.sync.dma_start(xt_sb[:, kt, :], xt_r[:, kt, :])

                # ---- q/k projections (transposed layout) ----
                for w_sb, raw in ((wq_sb, qraw), (wk_sb, kraw)):
                    for m in range(HPC):
                        for n in range(NSQ):
                            pp = ps.tile([P, SQW], fp32, tag="proj", bufs=2,
                                         name="pp")
                            for kt in range(KT):
                                nc.tensor.matmul(
                                    pp[:],
                                    w_sb[:, kt, m * HD:(m + 1) * HD],
                                    xt_sb[:, kt, n * SQW:(n + 1) * SQW],
                                    start=(kt == 0),
                                    stop=(kt == KT - 1),
                                )
                            nc.vector.tensor_copy(
                                raw[:, m, n * SQW:(n + 1) * SQW], pp[:]
                            )

                # ---- v projection (natural layout) ----
                for m in range(NT):
                    pv = ps.tile([P, DC], fp32, tag="proj", bufs=2, name="pv")
                    for kt in range(KT):
                        nc.tensor.matmul(
                            pv[:],
                            xt_sb[:, kt, m * P:(m + 1) * P],
                            wv_sb[:, kt, :],
                            start=(kt == 0),
                            stop=(kt == KT - 1),
                        )
                    nc.vector.tensor_copy(v_sb[:, m, :], pv[:])

                # ---- RoPE ----
                for raw, fin in ((qraw, qfin), (kraw, kfin)):
                    for m in range(HPC):
                        for n in range(NSQ):
                            nsl = slice(n * SQW, (n + 1) * SQW)
                            pr = ps.tile([P, SQW], fp32, tag="proj", bufs=2,
                                         name="pr")
                            nc.tensor.matmul(
                                pr[:], rt_sb[:], raw[:, m, nsl],
                                start=True, stop=True,
                            )
                            t1 = work.tile([P, SQW], fp32, tag="t1", bufs=2,
                                           name="t1")
                            t2 = work.tile([P, SQW], fp32, tag="t2", bufs=2,
                                           name="t2")
                            nc.vector.tensor_mul(t1[:], raw[:, m, nsl],
                                                 cos_sb[:, nsl])
                            nc.vector.tensor_mul(t2[:], pr[:], sin_sb[:, nsl])
                            nc.vector.tensor_add(fin[:, m, nsl], t1[:], t2[:])

            # gath pool opens in the SBUF space freed by xtp; the full Wo.T
            # load (8MB) overlaps the attention phase
            with tc.tile_pool(name=f"gath{r}", bufs=1) as gath:
                wo_sb = gath.tile([P, KT, D], bf16, name=f"wo_sb{r}")
                for kt in range(KT):
                    nc.sync.dma_start(wo_sb[:, kt, :], wo_r[:, kt, :])

                # ---- attention ----
                for h in range(HPC):
                    hsl = slice(h * HD, (h + 1) * HD)
                    for i in range(NSQ):
                        sq = slice(i * SQW, (i + 1) * SQW)
                        njt = 4 * i + 4
                        pa = ps.tile([HD, SQW], fp32, tag="attn", bufs=2,
                                     name="pa")
                        ssum = work.tile([P, SQW], fp32, tag="ssum", bufs=2,
                                         name="ssum")
                        for j in range(njt):
                            psc = ps.tile([P, SQW], fp32, tag="sc", bufs=2,
                                          name="psc")
                            nc.tensor.matmul(
                                psc[:],
                                kfin[:, h, j * P:(j + 1) * P],
                                qfin[:, h, sq],
                                start=True, stop=True,
                            )
                            e = work.tile([P, SQW], bf16, tag="e", bufs=3,
                                          name="e")
                            nc.scalar.activation(e[:], psc[:], Exp, scale=SM)
                            m = j - 4 * i
                            if m >= 0:
                                em = work.tile([P, SQW], bf16, tag="em",
                                               bufs=3, name="em")
                                nc.vector.tensor_mul(
                                    em[:], e[:],
                                    msk_sb[:, 384 - 128 * m: 896 - 128 * m]
                                )
                                e = em
                            if j == 0:
                                nc.vector.tensor_copy(ssum[:], e[:])
                            else:
                                nc.vector.tensor_add(ssum[:], ssum[:], e[:])
                            nc.tensor.matmul(
                                pa[:],
                                v_sb[:, j, hsl],
                                e[:],
                                start=(j == 0),
                                stop=(j == njt - 1),
                            )
                        sb = work.tile([P, SQW], bf16, tag="ssumb", bufs=2,
                                       name="sb")
                        nc.vector.tensor_copy(sb[:], ssum[:])
                        pl = ps.tile([P, SQW], fp32, tag="l", bufs=2,
                                     name="pl")
                        nc.tensor.matmul(pl[:], ones_sb[:], sb[:],
                                         start=True, stop=True)
                        rec = work.tile([P, SQW], fp32, tag="rec", bufs=2,
                                        name="rec")
                        nc.vector.reciprocal(rec[:], pl[:])
                        nc.vector.tensor_mul(attnT[:, h, sq], pa[:], rec[:])
                        # ship finished 512-wide chunk into the AllToAll
                        # staging buffer (2 dest cores per chunk)
                        for jj in (2 * i, 2 * i + 1):
                            nc.sync.dma_start(
                                a2a_in[jj, h * HD:(h + 1) * HD, :],
                                attnT[:, h, jj * SCS:(jj + 1) * SCS],
                            )

                nc.gpsimd.collective_compute(
                    "AllToAll",
                    mybir.AluOpType.bypass,
                    replica_groups=[list(range(NCORES))],
                    ins=[a2a_in[:].opt()],
                    outs=[a2a_out[:].opt()],
                )

                # ---- output projection (this core's 256 seq rows) ----
                # a2a_out[j] = attn.T[256j:256j+256 dattn rows, my seq slice]
                ag_sb = gath.tile([P, KT, SCS], bf16, name=f"ag_sb{r}")
                for j in range(NCORES):
                    for h in range(HPC):
                        nc.sync.dma_start(
                            ag_sb[:, 2 * j + h, :],
                            a2a_out[j, h * HD:(h + 1) * HD, :],
                        )
                out_sb = gath.tile([P, SCS // P, D], fp32, name=f"out_sb{r}")
                for m in range(SCS // P):
                    for n in range(NSQ):
                        po = ps.tile([P, SQW], fp32, tag="proj", bufs=2,
                                     name="po")
                        for kt in range(KT):
                            nc.tensor.matmul(
                                po[:],
                                ag_sb[:, kt, m * P:(m + 1) * P],
                                wo_sb[:, kt, n * SQW:(n + 1) * SQW],
                                start=(kt == 0),
                                stop=(kt == KT - 1),
                            )
                        nc.vector.tensor_copy(
                            out_sb[:, m, n * SQW:(n + 1) * SQW], po[:]
                        )
                    nc.sync.dma_start(out_r[m], out_sb[:, m, :])

    with tile.TileContext(nc) as tc:
        for rep in range(reps):
            emit_body(tc, rep)

    nc.compile()
    return nc


def _get_nc(reps=1):
    key = ("nc", reps)
    if key not in _NC_CACHE:
        _NC_CACHE[key] = _build_nc(reps)
    return _NC_CACHE[key]


def _host_tables():
    import ml_dtypes

    bf = ml_dtypes.bfloat16
    inv_freq = 1.0 / (10000.0 ** (np.arange(0, HD, 2, dtype=np.float32) / HD))
    t = np.arange(S, dtype=np.float32)
    freqs = np.outer(t, inv_freq)
    emb = np.concatenate([freqs, freqs], axis=-1)        # [S, HD]
    cosT = np.ascontiguousarray(np.cos(emb).T).astype(np.float32)
    sinT = np.ascontiguousarray(np.sin(emb).T).astype(np.float32)

    rt = np.zeros((HD, HD), dtype=np.float32)
    for e in range(64):
        rt[e, e + 64] = 1.0
    for e in range(64, HD):
        rt[e, e - 64] = -1.0

    y = np.arange(896)[None, :]
    tl = np.arange(P)[:, None]
    mskM = (tl <= (y - 384)).astype(np.float32)

    ones = np.ones((P, P), dtype=np.float32)
    return cosT, sinT, rt.astype(bf), mskM.astype(bf), ones.astype(bf)


def _prep_in_maps(hidden_states, Wq, Wk, Wv, Wo):
    import ml_dtypes

    bf = ml_dtypes.bfloat16
    X = np.asarray(hidden_states, dtype=np.float32).reshape(S, D)
    Wq = np.asarray(Wq, dtype=np.float32)
    Wk = np.asarray(Wk, dtype=np.float32)
    Wv = np.asarray(Wv, dtype=np.float32)
    Wo = np.asarray(Wo, dtype=np.float32)

    XT = np.ascontiguousarray(X.T).astype(bf)
    WoT = np.ascontiguousarray(Wo.T).astype(bf)
    cosT, sinT, rt, mskM, ones = _host_tables()

    in_maps = []
    for c in range(NCORES):
        sl = slice(DC * c, DC * (c + 1))
        in_maps.append({
            "xt": XT,
            "wq": np.ascontiguousarray(Wq[sl].T).astype(bf),
            "wk": np.ascontiguousarray(Wk[sl].T).astype(bf),
            "wv": np.ascontiguousarray(Wv[sl].T).astype(bf),
            "wo": WoT,
            "cost": cosT,
            "sint": sinT,
            "rt": rt,
            "msk": mskM,
            "ones": ones,
        })
    return in_maps


def kernel(hidden_states, Wq, Wk, Wv, Wo):
    global LAST_RESULTS
    from concourse.bass_utils import run_bass_kernel_spmd

    in_maps = _prep_in_maps(hidden_states, Wq, Wk, Wv, Wo)
    nc = _get_nc()
    res = run_bass_kernel_spmd(nc, in_maps, core_ids=list(range(NCORES)))
    LAST_RESULTS = res

    out = np.concatenate(
        [np.asarray(res.results[c]["out"]) for c in range(NCORES)], axis=0
    )
    return out.reshape(1, S, D).astype(np.float32)


# revision 41
# speedup vs baseline: 2.2061x; 1.2239x over previous
"""TRN2 Bass/Tile kernel: Llama attention block (B=1, S=2048, D=2048, H=16, causal).

Sharding: tensor-parallel over heads. 16 heads / 8 cores = 2 heads per core.
Wq/Wk/Wv column-sharded (256 dims per core), Wo column-sharded on the output
side after an AllGather of the per-core attention outputs.

Per-core dataflow (all matmuls bf16 operands, fp32 PSUM accumulate):
  - host passes X.T so the contraction dim is on partitions everywhere
  - qT/kT computed in [hd, S] layout, v in natural [S, hd] layout
  - RoPE: rotate-half via a +-1 permutation matmul, then q' = q*cos + rot*sin
  - attention computes scoresT[t, sq] = kT_tile.T @ qT, exp on ScalarE (no
    max-subtraction: |scaled scores| < 5 for this data), causal mask by
    multiplying a 0/1 staircase, probs consumed directly as the moving
    operand of the v-matmul -> attn.T[hd, sq] with zero transposes
  - softmax denominators: running DVE sum over t-tiles, then a ones-matmul
    broadcasts the partition-sum to all partitions; DVE reciprocal+mul
  - per-head AllGather of attn.T into all cores, column-sharded Wo matmul
"""

import os
import sys

import numpy as np

for _p in ("/opt/trn_rl_repo",):
    if _p not in sys.path and os.path.isdir(_p):
        sys.path.insert(0, _p)

P = 128            # SBUF partitions
S = 2048           # sequence length
D = 2048           # hidden dim
NCORES = 8
DC = D // NCORES   # 256 = head-dims per core
HPC = 2            # heads per core
HD = 128           # head dim
KT = D // P        # 16 contraction tiles
SQW = 512          # sq tile width (moving free dim)
NSQ = S // SQW     # 4
NT = S // P        # 16 t tiles
SCS = S // NCORES  # 256 output seq rows per core (sequence-parallel Wo)
SM = float(1.0 / np.sqrt(HD))

_NC_CACHE = {}
LAST_RESULTS = None


def _build_nc(reps=1):
    import concourse.bacc as bacc
    import concourse.mybir as mybir
    from concourse import tile

    fp32 = mybir.dt.float32
    bf16 = mybir.dt.bfloat16
    Exp = mybir.ActivationFunctionType.Exp

    nc = bacc.Bacc("TRN2", num_devices=NCORES, debug=False)

    xt = nc.dram_tensor("xt", [D, S], bf16, kind="ExternalInput")
    wq = nc.dram_tensor("wq", [D, DC], bf16, kind="ExternalInput")
    wk = nc.dram_tensor("wk", [D, DC], bf16, kind="ExternalInput")
    wv = nc.dram_tensor("wv", [D, DC], bf16, kind="ExternalInput")
    wo = nc.dram_tensor("wo", [D, D], bf16, kind="ExternalInput")  # full Wo.T
    cost = nc.dram_tensor("cost", [HD, S], fp32, kind="ExternalInput")
    sint = nc.dram_tensor("sint", [HD, S], fp32, kind="ExternalInput")
    rt = nc.dram_tensor("rt", [HD, HD], bf16, kind="ExternalInput")
    msk = nc.dram_tensor("msk", [P, 896], bf16, kind="ExternalInput")
    ones = nc.dram_tensor("ones", [P, P], bf16, kind="ExternalInput")
    out = nc.dram_tensor("out", [SCS, D], fp32, kind="ExternalOutput")

    xt_r = xt.rearrange("(k p) s -> p k s", p=P)
    wq_r = wq.rearrange("(k p) d -> p k d", p=P)
    wk_r = wk.rearrange("(k p) d -> p k d", p=P)
    wv_r = wv.rearrange("(k p) d -> p k d", p=P)
    wo_r = wo.rearrange("(k p) d -> p k d", p=P)
    out_r = out.rearrange("(m p) d -> m p d", p=P)

    def emit_body(tc, rep):
        r = f"r{rep}"
        with (
            tc.tile_pool(name=f"const{r}", bufs=1) as const,
            tc.tile_pool(name=f"acts{r}", bufs=1) as acts,
            tc.tile_pool(name=f"work{r}", bufs=2) as work,
            tc.tile_pool(name=f"ps{r}", bufs=2, space="PSUM") as ps,
            tc.tile_pool(name=f"dram{r}", bufs=1, space="DRAM") as dram,
        ):
            # ---- constants / weights ----
            wq_sb = const.tile([P, KT, DC], bf16, name=f"wq_sb{r}")
            wk_sb = const.tile([P, KT, DC], bf16, name=f"wk_sb{r}")
            wv_sb = const.tile([P, KT, DC], bf16, name=f"wv_sb{r}")
            cos_sb = const.tile([HD, S], fp32, name=f"cos_sb{r}")
            sin_sb = const.tile([HD, S], fp32, name=f"sin_sb{r}")
            rt_sb = const.tile([HD, HD], bf16, name=f"rt_sb{r}")
            msk_sb = const.tile([P, 896], bf16, name=f"msk_sb{r}")
            ones_sb = const.tile([P, P], bf16, name=f"ones_sb{r}")
            # q/k weights first: the first projection groups need them + xt
            nc.sync.dma_start(wq_sb[:], wq_r)
            nc.sync.dma_start(wk_sb[:], wk_r)

            # ---- persistent activations ----
            qfin = acts.tile([HD, HPC, S], bf16, name=f"qfin{r}")
            kfin = acts.tile([HD, HPC, S], bf16, name=f"kfin{r}")
            v_sb = acts.tile([P, NT, DC], bf16, name=f"v_sb{r}")
            attnT = acts.tile([HD, HPC, S], bf16, name=f"attnT{r}")

            # AllToAll buffers (one per head so head-0's exchange overlaps
            # head-1's attention): block j of a2a_in[h] (this core's attn.T
            # columns s in [256j, 256j+256)) is sent to core j; core j then
            # holds attn.T[:, its seq slice] from every core.
            a2a_in = [dram.tile([NCORES, HD, SCS], bf16, name=f"a2ain{h}{r}")
                      for h in range(HPC)]
            a2a_out = [dram.tile([NCORES, HD, SCS], bf16, name=f"a2aout{h}{r}")
                       for h in range(HPC)]

            with tc.tile_pool(name=f"xtp{r}", bufs=1) as xtp:
                xt_sb = xtp.tile([P, KT, S], bf16, name=f"xt_sb{r}")
                qraw = xtp.tile([HD, HPC, S], bf16, name=f"qraw{r}")
                kraw = xtp.tile([HD, HPC, S], bf16, name=f"kraw{r}")
                for kt in range(KT):
                    nc.sync.dma_start(xt_sb[:, kt, :], xt_r[:, kt, :])
                # remaining constants, needed later than wq/wk/xt
                nc.sync.dma_start(rt_sb[:], rt[:])
                nc.sync.dma_start(cos_sb[:], cost[:])
                nc.sync.dma_start(sin_sb[:], sint[:])
                nc.sync.dma_start(wv_sb[:], wv_r)
                nc.sync.dma_start(msk_sb[:], msk[:])
                nc.sync.dma_start(ones_sb[:], ones[:])

                def qk_proj(m):
                    for w_sb, raw in ((wq_sb, qraw), (wk_sb, kraw)):
                        for n in range(NSQ):
                            pp = ps.tile([P, SQW], fp32, tag="proj", bufs=2,
                                         name="pp")
                            for kt in range(KT):
                                nc.tensor.matmul(
                                    pp[:],
                                    w_sb[:, kt, m * HD:(m + 1) * HD],
                                    xt_sb[:, kt, n * SQW:(n + 1) * SQW],
                                    start=(kt == 0),
                                    stop=(kt == KT - 1),
                                )
                            nc.scalar.copy(
                                raw[:, m, n * SQW:(n + 1) * SQW], pp[:]
                            )

                def rope(m):
                    for raw, fin in ((qraw, qfin), (kraw, kfin)):
                        for n in range(NSQ):
                            nsl = slice(n * SQW, (n + 1) * SQW)
                            pr = ps.tile([P, SQW], fp32, tag="proj", bufs=2,
                                         name="pr")
                            nc.tensor.matmul(
                                pr[:], rt_sb[:], raw[:, m, nsl],
                                start=True, stop=True,
                            )
                            t1 = work.tile([P, SQW], fp32, tag="t1", bufs=2,
                                           name="t1")
                            t2 = work.tile([P, SQW], fp32, tag="t2", bufs=2,
                                           name="t2")
                            # t1 on GpSimd (otherwise idle) to shorten the
                            # per-slice DVE chain
                            nc.gpsimd.tensor_mul(t1[:], raw[:, m, nsl],
                                                 cos_sb[:, nsl])
                            nc.vector.tensor_mul(t2[:], pr[:], sin_sb[:, nsl])
                            nc.vector.tensor_add(fin[:, m, nsl], t1[:], t2[:])

                qk_proj(0)
                qk_proj(1)

                # ---- v projection (natural layout) ----
                for m in range(NT):
                    pv = ps.tile([P, DC], fp32, tag="proj", bufs=2, name="pv")
                    for kt in range(KT):
                        nc.tensor.matmul(
                            pv[:],
                            xt_sb[:, kt, m * P:(m + 1) * P],
                            wv_sb[:, kt, :],
                            start=(kt == 0),
                            stop=(kt == KT - 1),
                        )
                    nc.vector.tensor_copy(v_sb[:, m, :], pv[:])

                rope(0)
                rope(1)

            # gath pool opens in the SBUF space freed by xtp; the full Wo.T
            # load (8MB) overlaps the attention phase
            with tc.tile_pool(name=f"gath{r}", bufs=1) as gath:
                wo_sb = gath.tile([P, KT, D], bf16, name=f"wo_sb{r}")
                for kt in range(KT):
                    nc.sync.dma_start(wo_sb[:, kt, :], wo_r[:, kt, :])

                # ---- attention ----
                last_attn_mm = None
                last_exp = None
                for h in range(HPC):
                    hsl = slice(h * HD, (h + 1) * HD)
                    for i in range(NSQ):
                        sq = slice(i * SQW, (i + 1) * SQW)
                        njt = 4 * i + 4
                        pa = ps.tile([HD, SQW], fp32, tag="attn", bufs=2,
                                     name="pa")
                        # softmax denominators accumulate on the PE: an
                        # all-ones stationary operand broadcasts the
                        # partition-sum of each exp tile into every row
                        pl = ps.tile([P, SQW], fp32, tag="l", bufs=2,
                                     name="pl")
                        for j in range(njt):
                            # alternate between the sc banks and the proj
                            # banks (idle during attention) -> 4-deep
                            # score pipeline
                            psc = ps.tile([P, SQW], fp32,
                                          tag=("sc" if j % 2 else "proj"),
                                          bufs=2, name="psc")
                            nc.tensor.matmul(
                                psc[:],
                                kfin[:, h, j * P:(j + 1) * P],
                                qfin[:, h, sq],
                                start=True, stop=True,
                            )
                            e = work.tile([P, SQW], bf16, tag="e", bufs=4,
                                          name="e")
                            last_exp = nc.scalar.activation(
                                e[:], psc[:], Exp, scale=SM)
                            m = j - 4 * i
                            if m >= 0:
                                em = work.tile([P, SQW], bf16, tag="em",
                                               bufs=4, name="em")
                                nc.vector.tensor_mul(
                                    em[:], e[:],
                                    msk_sb[:, 384 - 128 * m: 896 - 128 * m]
                                )
                                e = em
                            nc.tensor.matmul(
                                pa[:],
                                v_sb[:, j, hsl],
                                e[:],
                                start=(j == 0),
                                stop=(j == njt - 1),
                            )
                            last_attn_mm = nc.tensor.matmul(
                                pl[:],
                                ones_sb[:],
                                e[:],
                                start=(j == 0),
                                stop=(j == njt - 1),
                            )
                        rec = work.tile([P, SQW], fp32, tag="rec", bufs=2,
                                        name="rec")
                        nc.vector.reciprocal(rec[:], pl[:])
                        nc.vector.tensor_mul(attnT[:, h, sq], pa[:], rec[:])
                        # ship finished 512-wide chunk into the AllToAll
                        # staging buffer (2 dest cores per chunk)
                        for jj in (2 * i, 2 * i + 1):
                            nc.sync.dma_start(
                                a2a_in[h][jj, :, :],
                                attnT[:, h, jj * SCS:(jj + 1) * SCS],
                            )
                    # exchange this head's attn.T while the next head computes
                    nc.gpsimd.collective_compute(
                        "AllToAll",
                        mybir.AluOpType.bypass,
                        replica_groups=[list(range(NCORES))],
                        ins=[a2a_in[h][:].opt()],
                        outs=[a2a_out[h][:].opt()],
                    )

                # ---- output projection (this core's 256 seq rows) ----
                # a2a_out[h][j] = attn.T rows of head (2j+h), my seq slice.
                # Gather-in DMAs ride the (idle) vector-engine DGE queue so
                # their wait on the collectives can't head-of-line-block the
                # sync queue that carries the a2a input stores.
                import bass_rust as _br
                ag_sb = gath.tile([P, KT, SCS], bf16, name=f"ag_sb{r}")
                for j in range(NCORES):
                    for h in range(HPC):
                        d = nc.scalar.dma_start(
                            ag_sb[:, 2 * j + h, :],
                            a2a_out[h][j, :, :],
                        )
                        # keep the collective-gated loads behind the last exp
                        # in the ACT queue so they can't head-of-line-block
                        # the attention activations
                        _br.add_dep_helper(d.ins, last_exp.ins, False,
                                           "ag after attention exps")
                out_sb = gath.tile([P, SCS // P, D], fp32, name=f"out_sb{r}")
                # Two accumulation passes per output tile: head-0 k-tiles
                # (available after the first AllToAll) then head-1 k-tiles.
                # 8 groups spread over all 8 PSUM banks so every pass-A half
                # runs while the second AllToAll is still in flight.
                grp_tags = [("proj", 2), ("proj", 2), ("sc", 2), ("sc", 2),
                            ("attn", 2), ("attn", 2), ("l", 2), ("l", 2)]
                mns = [(m, n) for m in range(SCS // P) for n in range(NSQ)]
                po_tiles = []
                for g, (m, n) in enumerate(mns):
                    tag, b = grp_tags[g]
                    po_tiles.append(
                        ps.tile([P, SQW], fp32, tag=tag, bufs=b, name=f"po{g}")
                    )
                for h in range(HPC):
                    for g, (m, n) in enumerate(mns):
                        for ki in range(NCORES):
                            kt = 2 * ki + h
                            mm = nc.tensor.matmul(
                                po_tiles[g][:],
                                ag_sb[:, kt, m * P:(m + 1) * P],
                                wo_sb[:, kt, n * SQW:(n + 1) * SQW],
                                start=(h == 0 and ki == 0),
                                stop=(h == HPC - 1 and ki == NCORES - 1),
                                skip_group_check=True,
                            )
                            if h == 0 and ki == 0:
                                # keep Wo matmuls behind the attention stream
                                # in the PE queue: they wait on the exchange,
                                # and scheduling them early would head-of-line
                                # block the remaining attention matmuls
                                _br.add_dep_helper(
                                    mm.ins, last_attn_mm.ins, False,
                                    "wo after attention on PE")
                for g, (m, n) in enumerate(mns):
                    nc.vector.tensor_copy(
                        out_sb[:, m, n * SQW:(n + 1) * SQW], po_tiles[g][:]
                    )
                for m in range(SCS // P):
                    nc.sync.dma_start(out_r[m], out_sb[:, m, :])

    with tile.TileContext(nc) as tc:
        for rep in range(reps):
            emit_body(tc, rep)

    nc.compile()
    return nc


def _get_nc(reps=1):
    key = ("nc", reps)
    if key not in _NC_CACHE:
        _NC_CACHE[key] = _build_nc(reps)
    return _NC_CACHE[key]


def _host_tables():
    import ml_dtypes

    bf = ml_dtypes.bfloat16
    inv_freq = 1.0 / (10000.0 ** (np.arange(0, HD, 2, dtype=np.float32) / HD))
    t = np.arange(S, dtype=np.float32)
    freqs = np.outer(t, inv_freq)
    emb = np.concatenate([freqs, freqs], axis=-1)        # [S, HD]
    cosT = np.ascontiguousarray(np.cos(emb).T).astype(np.float32)
    sinT = np.ascontiguousarray(np.sin(emb).T).astype(np.float32)

    rt = np.zeros((HD, HD), dtype=np.float32)
    for e in range(64):
        rt[e, e + 64] = 1.0
    for e in range(64, HD):
        rt[e, e - 64] = -1.0

    y = np.arange(896)[None, :]
    tl = np.arange(P)[:, None]
    mskM = (tl <= (y - 384)).astype(np.float32)

    ones = np.ones((P, P), dtype=np.float32)
    return cosT, sinT, rt.astype(bf), mskM.astype(bf), ones.astype(bf)


def _prep_in_maps(hidden_states, Wq, Wk, Wv, Wo):
    import ml_dtypes

    bf = ml_dtypes.bfloat16
    X = np.asarray(hidden_states, dtype=np.float32).reshape(S, D)
    Wq = np.asarray(Wq, dtype=np.float32)
    Wk = np.asarray(Wk, dtype=np.float32)
    Wv = np.asarray(Wv, dtype=np.float32)
    Wo = np.asarray(Wo, dtype=np.float32)

    XT = np.ascontiguousarray(X.T).astype(bf)
    WoT = np.ascontiguousarray(Wo.T).astype(bf)
    cosT, sinT, rt, mskM, ones = _host_tables()

    in_maps = []
    for c in range(NCORES):
        sl = slice(DC * c, DC * (c + 1))
        in_maps.append({
            "xt": XT,
            "wq": np.ascontiguousarray(Wq[sl].T).astype(bf),
            "wk": np.ascontiguousarray(Wk[sl].T).astype(bf),
            "wv": np.ascontiguousarray(Wv[sl].T).astype(bf),
            "wo": WoT,
            "cost": cosT,
            "sint": sinT,
            "rt": rt,
            "msk": mskM,
            "ones": ones,
        })
    return in_maps


def kernel(hidden_states, Wq, Wk, Wv, Wo):
    global LAST_RESULTS
    from concourse.bass_utils import run_bass_kernel_spmd

    in_maps = _prep_in_maps(hidden_states, Wq, Wk, Wv, Wo)
    nc = _get_nc()
    res = run_bass_kernel_spmd(nc, in_maps, core_ids=list(range(NCORES)))
    LAST_RESULTS = res

    out = np.concatenate(
        [np.asarray(res.results[c]["out"]) for c in range(NCORES)], axis=0
    )
    return out.reshape(1, S, D).astype(np.float32)
